# revision 17
# baseline (speedup 1.0000x reference)
"""BoxAnchorAssigner on 8 Trainium2 NeuronCores (Bass/Tile).

Strategy
--------
anchors form a regular (H=704, W=200, na=2) grid; the standup-IoU between an
anchor (i,j,d) and gt m separates:  inter = xe_d[i,m] * ye_d[j,m]  where
xe/ye are per-axis overlap extents.  With S[m] = a1 + a2[m],
iou = inter/(S-inter) = r/(1-r) monotone in r = inter/S, so ALL outputs
(thresholds, row argmax, column argmax "top1") can be computed in
R = xe * (ye*invS) space.

Sharding: the W=200 axis is split 25-per-core across 8 cores.  Each core
builds the tiny global xe/ye tables (so the per-gt top1 is available
locally without collectives) and sweeps only its own 50 (j,d) pairs x
6 i-tiles = 300 (128 anchor x 128 gt) tiles.

Per tile: PE computes R = xe_slice^T @ diag(ye'_jd) into PSUM (the
m-layout xe table is directly the transposed weights); the DVE then does
one tensor_reduce(max) and one scalar_tensor_tensor with accum_out that
computes sum((R>=Rmax)*(128-m)) = 128-argmax in a single pass.  Per-gt
encode rows (precomputed on host, 129x12, row 128 = zeros) are gathered
per-anchor with indirect DMA using the pos-masked argmax index; the top1
flags are delivered by an indirect-DMA scatter of the separable per-gt
argmax candidates.
"""

import numpy as np

import concourse.bass as bass
import concourse.mybir as mybir
from concourse.tile import TileContext
from concourse.bass_utils import run_bass_kernel_spmd

F32 = mybir.dt.float32
U32 = mybir.dt.uint32
OP = mybir.AluOpType
AX = mybir.AxisListType

H, W, NA, M = 704, 200, 2, 128
A = H * W * NA
NC_ = 8
WLOC = W // NC_            # 25 j's per core
JD = WLOC * NA             # 50 (j,d) pairs per core
NIT = 6                    # i-tiles (704 -> 5x128 + 64)
T = JD * NIT               # 300 tiles per core
NLOC = T * 128             # 38400 padded anchors per core
BIGI = float(2 ** 20)

POSR = float(np.float32(0.6) / np.float32(1.6))       # 0.375
NEGR = float(np.float32(0.45) / np.float32(1.45))     # 0.310344...
BIGIDX = 9.0e6

_CACHE = {}


def _build_nc(split=True):
    nc = bass.Bass("TRN2", target_bir_lowering=False, debug=False, num_devices=NC_)

    def inp(name, shape, dt=F32):
        return nc.declare_dram_parameter(name, shape, dt, isOutput=False)

    # ---- inputs (global unless noted) ----------------------------------
    gxmaxv = inp("gxmaxv", [128, 1])       # gt standup fields, m on partitions
    gxminv = inp("gxminv", [128, 1])
    gymaxv = inp("gymaxv", [128, 1])
    gyminv = inp("gyminv", [128, 1])
    invsv = inp("invsv", [128, 2])         # 1/S per (m, d)
    axmaxb = inp("axmaxb", [128, NA * H])  # anchor x fields, replicated rows
    axminb = inp("axminb", [128, NA * H])
    aymaxb = inp("aymaxb", [128, NA * W])  # anchor y fields (global)
    ayminb = inp("ayminb", [128, NA * W])
    aylmaxb = inp("aylmaxb", [128, JD])    # per-core: local y fields per (jl,d)
    aylminb = inp("aylminb", [128, JD])
    ident = inp("ident", [128, 128])       # identity matrix
    riota = inp("riota", [128, 128])       # 128 - m  along free
    ibig7 = inp("ibig7", [128, H])         # i - 2^20 along free
    ibig2 = inp("ibig2", [128, W])         # j - 2^20 along free
    jlov = inp("jlov", [128, 1])           # per-core: global j base
    xadv = inp("xadv", [128, 6])           # xa/diag per (p, it), pad 0
    yadv = inp("yadv", [128, WLOC])        # per-core: ya/diag per jl
    gtab = inp("gtab", [129, 12])          # per-gt encode table + zero row
    iotam = inp("iotam", [128, 1])         # m per partition

    labels_o = nc.declare_dram_parameter("labels_loc", [JD, H], F32, isOutput=True)
    reg_o = nc.declare_dram_parameter("reg_loc", [JD, H, 8], F32, isOutput=True)
    dir_o = nc.declare_dram_parameter("dir_loc", [JD, H, 4], F32, isOutput=True)

    flags = nc.dram_tensor("t1flags", [NLOC], F32)
    idxrow = nc.dram_tensor("idxrow", [NA * NIT, WLOC * 128], F32)

    with TileContext(nc) as tc:
        with tc.tile_pool(name="main", bufs=1) as pool, \
             tc.tile_pool(name="dj", bufs=2) as djp, \
             tc.tile_pool(name="ps", bufs=6, space="PSUM") as psum, \
             tc.tile_pool(name="gps", bufs=2, space="PSUM") as gpsum:

            def load(ap_in, shape, dt=F32):
                t = pool.tile(shape, dt, tag=ap_in.name)
                nc.sync.dma_start(t[:], ap_in[:])
                return t

            GXMAXV = load(gxmaxv, [128, 1]); GXMINV = load(gxminv, [128, 1])
            GYMAXV = load(gymaxv, [128, 1]); GYMINV = load(gyminv, [128, 1])
            INVSV = load(invsv, [128, 2])
            AXMAXB = load(axmaxb, [128, NA * H]); AXMINB = load(axminb, [128, NA * H])
            AYMAXB = load(aymaxb, [128, NA * W]); AYMINB = load(ayminb, [128, NA * W])
            AYLMAXB = load(aylmaxb, [128, JD]); AYLMINB = load(aylminb, [128, JD])
            IDENT = load(ident, [128, 128]); RIOTA = load(riota, [128, 128])
            IBIG7 = load(ibig7, [128, H]); IBIG2 = load(ibig2, [128, W])
            JLOV = load(jlov, [128, 1])
            XADV = load(xadv, [128, 6]); YADV = load(yadv, [128, WLOC])
            IOTAM = load(iotam, [128, 1])

            # ---------- m-layout extent tables ----------
            def ext_table(amax_ap, amin_ap, gmaxv, gminv, n, inv_d, tag):
                m1 = pool.tile([128, n], F32, tag=f"m1_{tag}")
                nc.vector.tensor_scalar(out=m1[:], in0=amax_ap, scalar1=gmaxv[:], scalar2=None, op0=OP.min)
                m2 = pool.tile([128, n], F32, tag=f"m2_{tag}")
                nc.vector.tensor_scalar(out=m2[:], in0=amin_ap, scalar1=gminv[:], scalar2=None, op0=OP.max)
                t1 = pool.tile([128, n], F32, tag=f"t_{tag}")
                nc.vector.tensor_tensor(out=t1[:], in0=m1[:], in1=m2[:], op=OP.subtract)
                nc.vector.tensor_scalar(out=t1[:], in0=t1[:], scalar1=1.0, scalar2=0.0, op0=OP.add, op1=OP.max)
                if inv_d is not None:
                    nc.vector.tensor_scalar(out=t1[:], in0=t1[:], scalar1=INVSV[:, inv_d:inv_d + 1], scalar2=None, op0=OP.mult)
                return t1

            # xe (raw, used as PE weights + for top1), ye' (with invS)
            XEM = [ext_table(AXMAXB[:, d * H:(d + 1) * H], AXMINB[:, d * H:(d + 1) * H],
                             GXMAXV, GXMINV, H, None, f"xe{d}") for d in range(NA)]
            YEM = [ext_table(AYMAXB[:, d * W:(d + 1) * W], AYMINB[:, d * W:(d + 1) * W],
                             GYMAXV, GYMINV, W, d, f"ye{d}") for d in range(NA)]
            # local ye' columns (bit-identical recompute on the local slice)
            YEL = ext_table(AYLMAXB[:], AYLMINB[:], GYMAXV, GYMINV, JD, None, "yel")
            for d in range(NA):
                nc.vector.tensor_scalar(out=YEL[:, d::NA], in0=YEL[:, d::NA], scalar1=INVSV[:, d:d + 1], scalar2=None, op0=OP.mult)

            # ---------- top1 (separable, global) ----------
            # first-argmax + max per (m, d) over i and j
            def fam(table, n, ibig, tag):
                mxv = pool.tile([128, 1], F32, tag=f"mx_{tag}")
                nc.vector.tensor_reduce(out=mxv[:], in_=table[:], axis=AX.X, op=OP.max)
                junk = pool.tile([128, n], F32, tag=f"jk_{tag}")
                mn = pool.tile([128, 1], F32, tag=f"mn_{tag}")
                nc.vector.scalar_tensor_tensor(out=junk[:], in0=table[:], scalar=mxv[:], in1=ibig, op0=OP.is_ge, op1=OP.mult, accum_out=None)
                nc.vector.tensor_reduce(out=mn[:], in_=junk[:], axis=AX.X, op=OP.min)
                idx = pool.tile([128, 1], F32, tag=f"ix_{tag}")
                nc.vector.tensor_scalar(out=idx[:], in0=mn[:], scalar1=BIGI, scalar2=None, op0=OP.add)
                return mxv, idx

            mxs, ixs, mys, jxs = [], [], [], []
            for d in range(NA):
                a, b = fam(XEM[d], H, IBIG7[:], f"x{d}")
                mxs.append(a); ixs.append(b)
                a, b = fam(YEM[d], W, IBIG2[:], f"y{d}")
                mys.append(a); jxs.append(b)

            def tiny(op, a, b, tag):
                t = pool.tile([128, 1], F32, tag=tag)
                nc.vector.tensor_tensor(out=t[:], in0=a[:], in1=b[:], op=op)
                return t

            P = [tiny(OP.mult, mxs[d], mys[d], f"P{d}") for d in range(NA)]
            gt1 = tiny(OP.is_gt, P[1], P[0], "gt1")
            eqp = tiny(OP.is_equal, P[1], P[0], "eqp")
            li = tiny(OP.is_lt, ixs[1], ixs[0], "li")
            ei = tiny(OP.is_equal, ixs[1], ixs[0], "ei")
            lj = tiny(OP.is_lt, jxs[1], jxs[0], "lj")
            t_a = tiny(OP.min, ei, lj, "t_a")
            tlt = tiny(OP.max, li, t_a, "tlt")
            t_b = tiny(OP.min, eqp, tlt, "t_b")
            use1 = tiny(OP.max, gt1, t_b, "use1")

            def blend(a1, a0, mask, tag):  # mask ? a1 : a0
                t1 = tiny(OP.subtract, a1, a0, tag + "_d")
                t2 = tiny(OP.mult, mask, t1, tag + "_m")
                return tiny(OP.add, a0, t2, tag)

            iT = blend(ixs[1], ixs[0], use1, "iT")
            jT = blend(jxs[1], jxs[0], use1, "jT")

            # Lidx = (iT%128)*T + ((jT-jlo)*2 + dT)*6 + iT//128 ; dT == use1
            iu = pool.tile([128, 1], U32, tag="iu")
            nc.vector.tensor_copy(iu[:], iT[:])
            pmu = pool.tile([128, 1], U32, tag="pmu")
            nc.vector.tensor_scalar(out=pmu[:], in0=iu[:], scalar1=127, scalar2=None, op0=OP.bitwise_and)
            itu = pool.tile([128, 1], U32, tag="itu")
            nc.vector.tensor_scalar(out=itu[:], in0=iu[:], scalar1=7, scalar2=None, op0=OP.logical_shift_right)
            pmf = pool.tile([128, 1], F32, tag="pmf")
            nc.vector.tensor_copy(pmf[:], pmu[:])
            itf = pool.tile([128, 1], F32, tag="itf")
            nc.vector.tensor_copy(itf[:], itu[:])
            l1 = tiny(OP.subtract, jT, JLOV, "l1")
            l2 = pool.tile([128, 1], F32, tag="l2")
            nc.vector.scalar_tensor_tensor(out=l2[:], in0=l1[:], scalar=2.0, in1=use1[:], op0=OP.mult, op1=OP.add)
            l2b = pool.tile([128, 1], F32, tag="l2b")
            nc.vector.scalar_tensor_tensor(out=l2b[:], in0=l2[:], scalar=6.0, in1=itf[:], op0=OP.mult, op1=OP.add)
            l3 = pool.tile([128, 1], F32, tag="l3")
            nc.vector.scalar_tensor_tensor(out=l3[:], in0=pmf[:], scalar=float(T), in1=l2b[:], op0=OP.mult, op1=OP.add)
            jhi = pool.tile([128, 1], F32, tag="jhi")
            nc.vector.tensor_scalar(out=jhi[:], in0=JLOV[:], scalar1=float(WLOC), scalar2=None, op0=OP.add)
            v1 = tiny(OP.is_ge, jT, JLOV, "v1")
            v2 = tiny(OP.is_lt, jT, jhi, "v2")
            vld = tiny(OP.min, v1, v2, "vld")
            big = pool.tile([128, 1], F32, tag="big")
            nc.any.memset(big[:], BIGIDX)
            lfin = blend(l3, big, vld, "lfin")
            lu = pool.tile([128, 1], U32, tag="lu")
            nc.vector.tensor_copy(lu[:], lfin[:])

            zt = pool.tile([128, T], F32, tag="zt")
            nc.any.memset(zt[:], 0.0)
            nc.sync.dma_start(flags[:].rearrange("(p f) -> p f", p=128), zt[:])
            onet = pool.tile([128, 1], F32, tag="onet")
            nc.any.memset(onet[:], 1.0)
            nc.gpsimd.indirect_dma_start(
                out=flags[:, None], out_offset=bass.IndirectOffsetOnAxis(ap=lu[:], axis=0),
                in_=onet[:], in_offset=None, bounds_check=NLOC - 1, oob_is_err=False)

            # ---------- diag tiles ----------
            DIAG = pool.tile([128, JD, 128], F32)
            for jd in range(JD):
                nc.vector.tensor_scalar(out=DIAG[:, jd, :], in0=IDENT[:], scalar1=YEL[:, jd:jd + 1], scalar2=None, op0=OP.mult)

            GTABS = pool.tile([128, 12], F32, tag="gtabs")
            nc.sync.dma_start(GTABS[:], gtab[0:128, :])
            FB = pool.tile([128, T], F32)
            nc.gpsimd.dma_start(FB[:], flags[:].rearrange("(p f) -> p f", p=128))

            # ---------- main sweep + pipelined gather, 12 chunks ----------
            POS = pool.tile([128, T], F32)
            LAB = pool.tile([128, T], F32)
            GTH = pool.tile([128, T, 12], F32)
            CT = WLOC  # 25 tiles per chunk
            for d in range(NA):
                for it in range(NIT):
                    off = d * NIT + it
                    npart = 128 if it < NIT - 1 else H - 128 * (NIT - 1)
                    lhs = XEM[d][:, it * 128:it * 128 + npart]
                    MXc = djp.tile([128, CT], F32, tag="MXc")
                    ACCc = djp.tile([128, CT], F32, tag="ACCc")
                    nc.any.memset(MXc[:], -1.0)
                    nc.any.memset(ACCc[:], 0.0)
                    for jl in range(WLOC):
                        jd = jl * 2 + d
                        R = psum.tile([128, 128], F32, tag="R")
                        nc.tensor.matmul(R[0:npart, :], lhsT=lhs, rhs=DIAG[:, jd, :], start=True, stop=True)
                        nc.vector.tensor_reduce(out=MXc[0:npart, jl:jl + 1], in_=R[0:npart, :], axis=AX.X, op=OP.max)
                        junk = djp.tile([128, 128], F32, tag="junk")
                        nc.vector.scalar_tensor_tensor(
                            out=junk[0:npart, :], in0=R[0:npart, :], scalar=MXc[0:npart, jl:jl + 1],
                            in1=RIOTA[0:npart, :], op0=OP.is_ge, op1=OP.mult,
                            accum_out=ACCc[0:npart, jl:jl + 1])
                    # ---- labels / pos for this chunk (t = jl*12 + off) ----
                    fbs = FB[:].rearrange("p (jl q) -> p jl q", q=12)[:, :, off]
                    poss = POS[:].rearrange("p (jl q) -> p jl q", q=12)[:, :, off]
                    labs = LAB[:].rearrange("p (jl q) -> p jl q", q=12)[:, :, off]
                    posa = djp.tile([128, CT], F32, tag="posa")
                    nc.vector.tensor_scalar(out=posa[:], in0=MXc[:], scalar1=POSR, scalar2=None, op0=OP.is_gt)
                    nc.vector.tensor_tensor(out=poss, in0=posa[:], in1=fbs, op=OP.max)
                    negc = djp.tile([128, CT], F32, tag="negc")
                    nc.vector.tensor_scalar(out=negc[:], in0=MXc[:], scalar1=NEGR, scalar2=None, op0=OP.is_lt)
                    x1 = djp.tile([128, CT], F32, tag="x1c")
                    nc.vector.tensor_tensor(out=x1[:], in0=negc[:], in1=poss, op=OP.mult)
                    nc.vector.tensor_tensor(out=x1[:], in0=negc[:], in1=x1[:], op=OP.subtract)
                    nc.vector.scalar_tensor_tensor(out=labs, in0=poss, scalar=2.0, in1=x1[:], op0=OP.mult, op1=OP.add)
                    nc.vector.tensor_scalar(out=labs, in0=labs, scalar1=-1.0, scalar2=None, op0=OP.add)
                    # ---- masked gather index ----
                    idxfc = djp.tile([128, CT], F32, tag="idxfc")
                    nc.vector.tensor_scalar(out=idxfc[:], in0=ACCc[:], scalar1=-1.0, scalar2=None, op0=OP.mult)
                    nc.vector.tensor_tensor(out=idxfc[:], in0=idxfc[:], in1=poss, op=OP.mult)
                    nc.vector.tensor_scalar(out=idxfc[:], in0=idxfc[:], scalar1=128.0, scalar2=None, op0=OP.add)
                    # ---- flatten -> DRAM, broadcast back, one-hot, PE gather ----
                    nc.sync.dma_start(idxrow[off, :].rearrange("(jl p) -> p jl", p=128), idxfc[:])
                    IDXB = djp.tile([128, CT * 128], F32, tag="IDXB")
                    nc.sync.dma_start(IDXB[:], idxrow[off:off + 1, :].to_broadcast([128, CT * 128]))
                    EQT = djp.tile([128, CT * 128], F32, tag="EQT")
                    nc.gpsimd.tensor_scalar(out=EQT[:], in0=IDXB[:], scalar1=IOTAM[:], scalar2=None, op0=OP.is_equal)
                    for g0 in range(0, CT, 4):
                        gn = min(4, CT - g0)
                        gp = gpsum.tile([128, 48], F32, tag="gp")
                        for k in range(gn):
                            nc.tensor.matmul(gp[:, k * 12:(k + 1) * 12], lhsT=EQT[:, (g0 + k) * 128:(g0 + k + 1) * 128], rhs=GTABS[:], start=True, stop=True)
                        gdst = GTH[:].rearrange("p (jl q) c -> p jl q c", q=12)[:, g0:g0 + gn, off, :]
                        nc.scalar.copy(gdst, gp[:, 0:gn * 12].rearrange("p (jl c) -> p jl c", c=12))

            # ---------- encode adjustments (cols 0,1,6,7), pos-masked ----------
            for it in range(NIT):
                g0 = GTH[:, :, 0].rearrange("p (jd nit) -> p jd nit", nit=NIT)[:, :, it]
                ps = POS[:].rearrange("p (jd nit) -> p jd nit", nit=NIT)[:, :, it]
                nc.vector.scalar_tensor_tensor(out=g0, in0=g0, scalar=XADV[:, it:it + 1], in1=ps, op0=OP.subtract, op1=OP.mult)
            for jl in range(WLOC):
                g1 = GTH[:, jl * 12:(jl + 1) * 12, 1]
                ps = POS[:, jl * 12:(jl + 1) * 12]
                nc.vector.scalar_tensor_tensor(out=g1, in0=g1, scalar=YADV[:, jl:jl + 1], in1=ps, op0=OP.subtract, op1=OP.mult)
            for d in range(NA):
                for col in (6, 7):
                    g = GTH[:, :, col].rearrange("p (jl d nit) -> p jl d nit", d=NA, nit=NIT)[:, :, d, :]
                    ps = POS[:].rearrange("p (jl d nit) -> p jl d nit", d=NA, nit=NIT)[:, :, d, :]
                    cc = _TRIG[d][0] if col == 6 else _TRIG[d][1]
                    nc.vector.scalar_tensor_tensor(out=g, in0=g, scalar=float(cc), in1=ps, op0=OP.subtract, op1=OP.mult)

            # ---------- outputs ----------
            for it in range(NIT):
                np_ = 128 if it < NIT - 1 else H - 128 * (NIT - 1)
                lsrc = LAB[:].rearrange("p (jd nit) -> p jd nit", nit=NIT)[0:np_, :, it]
                nc.sync.dma_start(labels_o[:, it * 128:it * 128 + np_].rearrange("jd i -> i jd"), lsrc)
                gsrc = GTH[:].rearrange("p (jd nit) c -> p jd nit c", nit=NIT)[0:np_, :, it, 0:8]
                nc.sync.dma_start(reg_o[:, it * 128:it * 128 + np_, :].rearrange("jd i c -> i jd c"), gsrc)
                dsrc = GTH[:].rearrange("p (jd nit) c -> p jd nit c", nit=NIT)[0:np_, :, it, 8:12]
                nc.sync.dma_start(dir_o[:, it * 128:it * 128 + np_, :].rearrange("jd i c -> i jd c"), dsrc)

    if split:
        _split_excess_waits(nc)
    return nc


def _split_excess_waits(nc, limit=1):
    """This walrus build accepts only `limit` sync-waits per instruction.
    Peel extras onto preceding same-engine wait carriers."""
    for f in nc.m.functions:
        for bb in f.blocks:
            insts = list(bb.instructions)
            out, changed, k = [], False, 0
            for inst in insts:
                si = inst.sync_info
                if (si is not None and si.on_wait is not None
                        and len(si.on_wait) > limit):
                    waits = list(si.on_wait)
                    for w in waits[:-limit]:
                        nop = mybir.InstDrain(name=f"{inst.name}-wsp{k}")
                        k += 1
                        nop.engine = inst.engine
                        nop.sync_info = mybir.SyncInfo(on_wait=[w], on_update=[])
                        out.append(nop)
                    si.on_wait = waits[-limit:]
                    inst.sync_info = si
                    changed = True
                out.append(inst)
            if changed:
                bb.instructions = out


def _trig():
    rs = np.deg2rad(np.asarray([0.0, 90.0], np.float32))
    return [(np.cos(rs[d:d + 1])[0], np.sin(rs[d:d + 1])[0]) for d in range(NA)]


_TRIG = _trig()


def _host_inputs(gt_boxes, anchors, standup_anchors):
    f = np.float32
    gt = np.asarray(gt_boxes, f)
    su = np.asarray(standup_anchors, f)
    an = np.asarray(anchors, f)

    # gt standup boxes (reference boxes3d_to_standup_bboxes in f32)
    x, y = gt[:, 0], gt[:, 1]
    l, w, r = gt[:, 3], gt[:, 4], gt[:, 6]
    c, s = np.abs(np.cos(r)), np.abs(np.sin(r))
    ex = f(0.5) * (l * c + w * s)
    ey = f(0.5) * (l * s + w * c)
    gxmin, gymin, gxmax, gymax = x - ex, y - ey, x + ex, y + ey
    a2 = (gxmax - gxmin + f(1)) * (gymax - gymin + f(1))

    # anchor standup fields from the actual input (i: stride 400; j: stride 2)
    axmax = np.stack([su[np.arange(H) * (W * NA) + d, 2] for d in range(NA)])
    axmin = np.stack([su[np.arange(H) * (W * NA) + d, 0] for d in range(NA)])
    aymax = np.stack([su[np.arange(W) * NA + d, 3] for d in range(NA)])
    aymin = np.stack([su[np.arange(W) * NA + d, 1] for d in range(NA)])
    a1c = np.array([(su[d, 2] - su[d, 0] + f(1)) * (su[d, 3] - su[d, 1] + f(1))
                    for d in range(NA)], f)
    invs = np.stack([f(1) / (a1c[d] + a2) for d in range(NA)], axis=1)

    # encode table
    la, wa, ha = an[0, 3], an[0, 4], an[0, 5]
    diag = np.sqrt(la * la + wa * wa)
    zg, lg, wg, hg, rg = gt[:, 2], gt[:, 3], gt[:, 4], gt[:, 5], gt[:, 6]
    two_pi = f(2.0 * np.pi)
    half_pi = f(np.pi / 2.0)
    q = np.clip(np.floor(np.mod(rg, two_pi) / half_pi), 0, 3).astype(np.int32)
    oh = np.zeros((M, 4), f)
    oh[np.arange(M), q] = f(1)
    gtab = np.zeros((129, 12), f)
    gtab[:M, 0] = gt[:, 0] / diag
    gtab[:M, 1] = gt[:, 1] / diag
    gtab[:M, 2] = (zg - f(-1.0)) / ha
    gtab[:M, 3] = np.log(lg / la)
    gtab[:M, 4] = np.log(wg / wa)
    gtab[:M, 5] = np.log(hg / ha)
    gtab[:M, 6] = np.cos(rg)
    gtab[:M, 7] = np.sin(rg)
    gtab[:M, 8:12] = oh

    xa = an[np.arange(H) * (W * NA), 0]
    ya = an[np.arange(W) * NA, 1]

    rep = lambda row: np.broadcast_to(np.asarray(row, f)[None, :], (128, len(row))).copy()
    col = lambda v: np.asarray(v, f).reshape(-1, 1).copy()

    xadv = np.zeros((128, 6), f)
    for it in range(NIT):
        n = min(128, H - it * 128)
        xadv[:n, it] = xa[it * 128:it * 128 + n] / diag

    glob = dict(
        gxmaxv=col(gxmax), gxminv=col(gxmin), gymaxv=col(gymax), gyminv=col(gymin),
        invsv=invs,
        axmaxb=rep(axmax.reshape(-1)), axminb=rep(axmin.reshape(-1)),
        aymaxb=rep(aymax.reshape(-1)), ayminb=rep(aymin.reshape(-1)),
        ident=np.eye(128, dtype=f),
        riota=rep(128.0 - np.arange(128)),
        ibig7=rep(np.arange(H) - BIGI),
        ibig2=rep(np.arange(W) - BIGI),
        gtab=gtab, xadv=xadv,
        iotam=np.arange(128, dtype=f).reshape(128, 1),
    )
    per_core = []
    for cidx in range(NC_):
        jlo = cidx * WLOC
        aylmax = np.zeros(JD, f)
        aylmin = np.zeros(JD, f)
        for jl in range(WLOC):
            for d in range(NA):
                aylmax[jl * 2 + d] = aymax[d, jlo + jl]
                aylmin[jl * 2 + d] = aymin[d, jlo + jl]
        m = dict(glob)
        m.update(aylmaxb=rep(aylmax), aylminb=rep(aylmin),
                 jlov=np.full((128, 1), f(jlo)),
                 yadv=rep(ya[jlo:jlo + WLOC] / diag))
        per_core.append(m)
    return per_core, None


def kernel(gt_boxes, anchors, standup_anchors):
    if "nc" not in _CACHE:
        _CACHE["nc"] = _build_nc()
    nc = _CACHE["nc"]

    per_core, _ = _host_inputs(gt_boxes, anchors, standup_anchors)
    res = run_bass_kernel_spmd(nc, per_core, list(range(NC_)))

    f = np.float32
    labels = np.empty((H, W, NA), f)
    reg = np.empty((H, W, NA, 8), f)
    dire = np.empty((H, W, NA, 4), f)
    for cidx in range(NC_):
        jlo = cidx * WLOC
        r = res.results[cidx]
        labels[:, jlo:jlo + WLOC, :] = r["labels_loc"].reshape(WLOC, NA, H).transpose(2, 0, 1)
        reg[:, jlo:jlo + WLOC, :, :] = r["reg_loc"].reshape(WLOC, NA, H, 8).transpose(2, 0, 1, 3)
        dire[:, jlo:jlo + WLOC, :, :] = r["dir_loc"].reshape(WLOC, NA, H, 4).transpose(2, 0, 1, 3)
    return labels.reshape(-1), reg.reshape(-1, 8), dire.reshape(-1, 4)


# revision 18
# speedup vs baseline: 1.5429x; 1.5429x over previous
"""BoxAnchorAssigner on 8 Trainium2 NeuronCores (Bass/Tile).

Strategy
--------
anchors form a regular (H=704, W=200, na=2) grid; the standup-IoU between an
anchor (i,j,d) and gt m separates:  inter = xe_d[i,m] * ye_d[j,m]  where
xe/ye are per-axis overlap extents.  With S[m] = a1 + a2[m],
iou = inter/(S-inter) = r/(1-r) monotone in r = inter/S, so ALL outputs
(thresholds, row argmax, column argmax "top1") can be computed in
R = xe * (ye*invS) space.

Sharding: the W=200 axis is split 25-per-core across 8 cores.  Each core
builds the tiny global xe/ye tables (so the per-gt top1 is available
locally without collectives) and sweeps only its own 50 (j,d) pairs x
6 i-tiles = 300 (128 anchor x 128 gt) tiles.

Per tile: PE computes R = xe_slice^T @ diag(ye'_jd) into PSUM (the
m-layout xe table is directly the transposed weights); the DVE then does
one tensor_reduce(max) and one scalar_tensor_tensor with accum_out that
computes sum((R>=Rmax)*(128-m)) = 128-argmax in a single pass.  Per-gt
encode rows (precomputed on host, 129x12, row 128 = zeros) are gathered
per-anchor with indirect DMA using the pos-masked argmax index; the top1
flags are delivered by an indirect-DMA scatter of the separable per-gt
argmax candidates.
"""

import numpy as np

import concourse.bass as bass
import concourse.mybir as mybir
from concourse.tile import TileContext
from concourse.bass_utils import run_bass_kernel_spmd

F32 = mybir.dt.float32
U32 = mybir.dt.uint32
OP = mybir.AluOpType
AX = mybir.AxisListType

H, W, NA, M = 704, 200, 2, 128
A = H * W * NA
NC_ = 8
WLOC = W // NC_            # 25 j's per core
JD = WLOC * NA             # 50 (j,d) pairs per core
NIT = 6                    # i-tiles (704 -> 5x128 + 64)
T = JD * NIT               # 300 tiles per core
NLOC = T * 128             # 38400 padded anchors per core
BIGI = float(2 ** 20)

POSR = float(np.float32(0.6) / np.float32(1.6))       # 0.375
NEGR = float(np.float32(0.45) / np.float32(1.45))     # 0.310344...
BIGIDX = 9.0e6

_CACHE = {}


def _build_nc(split=True):
    nc = bass.Bass("TRN2", target_bir_lowering=False, debug=False, num_devices=NC_)

    def inp(name, shape, dt=F32):
        return nc.declare_dram_parameter(name, shape, dt, isOutput=False)

    # ---- inputs (global unless noted) ----------------------------------
    gxmaxv = inp("gxmaxv", [128, 1])       # gt standup fields, m on partitions
    gxminv = inp("gxminv", [128, 1])
    gymaxv = inp("gymaxv", [128, 1])
    gyminv = inp("gyminv", [128, 1])
    invsv = inp("invsv", [128, 2])         # 1/S per (m, d)
    axmaxb = inp("axmaxb", [128, NA * H])  # anchor x fields, replicated rows
    axminb = inp("axminb", [128, NA * H])
    aymaxb = inp("aymaxb", [128, NA * W])  # anchor y fields (global)
    ayminb = inp("ayminb", [128, NA * W])
    aylmaxb = inp("aylmaxb", [128, JD])    # per-core: local y fields per (jl,d)
    aylminb = inp("aylminb", [128, JD])
    axmaxv = inp("axmaxv", [128, 12])      # anchor x field per (p, d*6+it), pad -1e9
    axminv = inp("axminv", [128, 12])      # pad +1e9
    gxmaxb = inp("gxmaxb", [128, 128])     # gt x fields replicated across partitions
    gxminb = inp("gxminb", [128, 128])
    riota = inp("riota", [128, 128])       # 128 - m  along free
    ibig7 = inp("ibig7", [128, H])         # i - 2^20 along free
    ibig2 = inp("ibig2", [128, W])         # j - 2^20 along free
    jlov = inp("jlov", [128, 1])           # per-core: global j base
    xadv = inp("xadv", [128, 6])           # xa/diag per (p, it), pad 0
    yadv = inp("yadv", [128, WLOC])        # per-core: ya/diag per jl
    gtab = inp("gtab", [129, 12])          # per-gt encode table + zero row
    iotam = inp("iotam", [128, 1])         # m per partition

    labels_o = nc.declare_dram_parameter("labels_loc", [JD, H], F32, isOutput=True)
    reg_o = nc.declare_dram_parameter("reg_loc", [JD, H, 8], F32, isOutput=True)
    dir_o = nc.declare_dram_parameter("dir_loc", [JD, H, 4], F32, isOutput=True)

    flags = nc.dram_tensor("t1flags", [NLOC], F32)
    idxrow = nc.dram_tensor("idxrow", [NA * NIT, WLOC * 128], F32)

    with TileContext(nc) as tc:
        with tc.tile_pool(name="main", bufs=1) as pool, \
             tc.tile_pool(name="dj", bufs=2) as djp, \
             tc.tile_pool(name="gps", bufs=4, space="PSUM") as gpsum:

            def load(ap_in, shape, dt=F32):
                t = pool.tile(shape, dt, tag=ap_in.name)
                nc.sync.dma_start(t[:], ap_in[:])
                return t

            GXMAXV = load(gxmaxv, [128, 1]); GXMINV = load(gxminv, [128, 1])
            GYMAXV = load(gymaxv, [128, 1]); GYMINV = load(gyminv, [128, 1])
            INVSV = load(invsv, [128, 2])
            AXMAXB = load(axmaxb, [128, NA * H]); AXMINB = load(axminb, [128, NA * H])
            AYMAXB = load(aymaxb, [128, NA * W]); AYMINB = load(ayminb, [128, NA * W])
            AYLMAXB = load(aylmaxb, [128, JD]); AYLMINB = load(aylminb, [128, JD])
            AXMAXV = load(axmaxv, [128, 12]); AXMINV = load(axminv, [128, 12])
            GXMAXB = load(gxmaxb, [128, 128]); GXMINB = load(gxminb, [128, 128])
            RIOTA = load(riota, [128, 128])
            IBIG7 = load(ibig7, [128, H]); IBIG2 = load(ibig2, [128, W])
            JLOV = load(jlov, [128, 1])
            XADV = load(xadv, [128, 6]); YADV = load(yadv, [128, WLOC])
            IOTAM = load(iotam, [128, 1])

            # ---------- m-layout extent tables ----------
            def ext_table(amax_ap, amin_ap, gmaxv, gminv, n, inv_d, tag):
                m1 = pool.tile([128, n], F32, tag=f"m1_{tag}")
                nc.vector.tensor_scalar(out=m1[:], in0=amax_ap, scalar1=gmaxv[:], scalar2=None, op0=OP.min)
                m2 = pool.tile([128, n], F32, tag=f"m2_{tag}")
                nc.vector.tensor_scalar(out=m2[:], in0=amin_ap, scalar1=gminv[:], scalar2=None, op0=OP.max)
                t1 = pool.tile([128, n], F32, tag=f"t_{tag}")
                nc.vector.tensor_tensor(out=t1[:], in0=m1[:], in1=m2[:], op=OP.subtract)
                nc.vector.tensor_scalar(out=t1[:], in0=t1[:], scalar1=1.0, scalar2=0.0, op0=OP.add, op1=OP.max)
                if inv_d is not None:
                    nc.vector.tensor_scalar(out=t1[:], in0=t1[:], scalar1=INVSV[:, inv_d:inv_d + 1], scalar2=None, op0=OP.mult)
                return t1

            # xe (raw, used as PE weights + for top1), ye' (with invS)
            XEM = [ext_table(AXMAXB[:, d * H:(d + 1) * H], AXMINB[:, d * H:(d + 1) * H],
                             GXMAXV, GXMINV, H, None, f"xe{d}") for d in range(NA)]
            YEM = [ext_table(AYMAXB[:, d * W:(d + 1) * W], AYMINB[:, d * W:(d + 1) * W],
                             GYMAXV, GYMINV, W, d, f"ye{d}") for d in range(NA)]
            # local ye' columns (bit-identical recompute on the local slice)
            YEL = ext_table(AYLMAXB[:], AYLMINB[:], GYMAXV, GYMINV, JD, None, "yel")
            for d in range(NA):
                nc.vector.tensor_scalar(out=YEL[:, d::NA], in0=YEL[:, d::NA], scalar1=INVSV[:, d:d + 1], scalar2=None, op0=OP.mult)

            # ---------- top1 (separable, global) ----------
            # first-argmax + max per (m, d) over i and j
            def fam(table, n, ibig, tag):
                mxv = pool.tile([128, 1], F32, tag=f"mx_{tag}")
                nc.vector.tensor_reduce(out=mxv[:], in_=table[:], axis=AX.X, op=OP.max)
                junk = pool.tile([128, n], F32, tag=f"jk_{tag}")
                mn = pool.tile([128, 1], F32, tag=f"mn_{tag}")
                nc.vector.scalar_tensor_tensor(out=junk[:], in0=table[:], scalar=mxv[:], in1=ibig, op0=OP.is_ge, op1=OP.mult, accum_out=None)
                nc.vector.tensor_reduce(out=mn[:], in_=junk[:], axis=AX.X, op=OP.min)
                idx = pool.tile([128, 1], F32, tag=f"ix_{tag}")
                nc.vector.tensor_scalar(out=idx[:], in0=mn[:], scalar1=BIGI, scalar2=None, op0=OP.add)
                return mxv, idx

            mxs, ixs, mys, jxs = [], [], [], []
            for d in range(NA):
                a, b = fam(XEM[d], H, IBIG7[:], f"x{d}")
                mxs.append(a); ixs.append(b)
                a, b = fam(YEM[d], W, IBIG2[:], f"y{d}")
                mys.append(a); jxs.append(b)

            def tiny(op, a, b, tag):
                t = pool.tile([128, 1], F32, tag=tag)
                nc.vector.tensor_tensor(out=t[:], in0=a[:], in1=b[:], op=op)
                return t

            P = [tiny(OP.mult, mxs[d], mys[d], f"P{d}") for d in range(NA)]
            gt1 = tiny(OP.is_gt, P[1], P[0], "gt1")
            eqp = tiny(OP.is_equal, P[1], P[0], "eqp")
            li = tiny(OP.is_lt, ixs[1], ixs[0], "li")
            ei = tiny(OP.is_equal, ixs[1], ixs[0], "ei")
            lj = tiny(OP.is_lt, jxs[1], jxs[0], "lj")
            t_a = tiny(OP.min, ei, lj, "t_a")
            tlt = tiny(OP.max, li, t_a, "tlt")
            t_b = tiny(OP.min, eqp, tlt, "t_b")
            use1 = tiny(OP.max, gt1, t_b, "use1")

            def blend(a1, a0, mask, tag):  # mask ? a1 : a0
                t1 = tiny(OP.subtract, a1, a0, tag + "_d")
                t2 = tiny(OP.mult, mask, t1, tag + "_m")
                return tiny(OP.add, a0, t2, tag)

            iT = blend(ixs[1], ixs[0], use1, "iT")
            jT = blend(jxs[1], jxs[0], use1, "jT")

            # Lidx = (iT%128)*T + ((jT-jlo)*2 + dT)*6 + iT//128 ; dT == use1
            iu = pool.tile([128, 1], U32, tag="iu")
            nc.vector.tensor_copy(iu[:], iT[:])
            pmu = pool.tile([128, 1], U32, tag="pmu")
            nc.vector.tensor_scalar(out=pmu[:], in0=iu[:], scalar1=127, scalar2=None, op0=OP.bitwise_and)
            itu = pool.tile([128, 1], U32, tag="itu")
            nc.vector.tensor_scalar(out=itu[:], in0=iu[:], scalar1=7, scalar2=None, op0=OP.logical_shift_right)
            pmf = pool.tile([128, 1], F32, tag="pmf")
            nc.vector.tensor_copy(pmf[:], pmu[:])
            itf = pool.tile([128, 1], F32, tag="itf")
            nc.vector.tensor_copy(itf[:], itu[:])
            l1 = tiny(OP.subtract, jT, JLOV, "l1")
            l2 = pool.tile([128, 1], F32, tag="l2")
            nc.vector.scalar_tensor_tensor(out=l2[:], in0=l1[:], scalar=2.0, in1=use1[:], op0=OP.mult, op1=OP.add)
            l2b = pool.tile([128, 1], F32, tag="l2b")
            nc.vector.scalar_tensor_tensor(out=l2b[:], in0=l2[:], scalar=6.0, in1=itf[:], op0=OP.mult, op1=OP.add)
            l3 = pool.tile([128, 1], F32, tag="l3")
            nc.vector.scalar_tensor_tensor(out=l3[:], in0=pmf[:], scalar=float(T), in1=l2b[:], op0=OP.mult, op1=OP.add)
            jhi = pool.tile([128, 1], F32, tag="jhi")
            nc.vector.tensor_scalar(out=jhi[:], in0=JLOV[:], scalar1=float(WLOC), scalar2=None, op0=OP.add)
            v1 = tiny(OP.is_ge, jT, JLOV, "v1")
            v2 = tiny(OP.is_lt, jT, jhi, "v2")
            vld = tiny(OP.min, v1, v2, "vld")
            big = pool.tile([128, 1], F32, tag="big")
            nc.any.memset(big[:], BIGIDX)
            lfin = blend(l3, big, vld, "lfin")
            lu = pool.tile([128, 1], U32, tag="lu")
            nc.vector.tensor_copy(lu[:], lfin[:])

            zt = pool.tile([128, T], F32, tag="zt")
            nc.any.memset(zt[:], 0.0)
            nc.sync.dma_start(flags[:].rearrange("(p f) -> p f", p=128), zt[:])
            onet = pool.tile([128, 1], F32, tag="onet")
            nc.any.memset(onet[:], 1.0)
            nc.gpsimd.indirect_dma_start(
                out=flags[:, None], out_offset=bass.IndirectOffsetOnAxis(ap=lu[:], axis=0),
                in_=onet[:], in_offset=None, bounds_check=NLOC - 1, oob_is_err=False)

            # ---------- ye rows -> DRAM -> broadcast YEB ----------
            ytd = nc.dram_tensor("ytd", [JD, 128], F32)
            nc.sync.dma_start(ytd[:].rearrange("jd m -> m jd"), YEL[:])
            YEB = pool.tile([128, JD * 128], F32)
            nc.sync.dma_start(YEB[:], ytd[:].rearrange("jd m -> (jd m)")[None, :].to_broadcast([128, JD * 128]))

            # ---------- XE tiles in (i-part, m-free) layout ----------
            XE = pool.tile([128, 12, 128], F32)
            for d in range(NA):
                for it in range(NIT):
                    k = d * 6 + it
                    xm1 = djp.tile([128, 128], F32, tag="xm1")
                    nc.vector.tensor_scalar(out=xm1[:], in0=GXMAXB[:], scalar1=AXMAXV[:, k:k + 1], scalar2=None, op0=OP.min)
                    xm2 = djp.tile([128, 128], F32, tag="xm2")
                    nc.vector.tensor_scalar(out=xm2[:], in0=GXMINB[:], scalar1=AXMINV[:, k:k + 1], scalar2=None, op0=OP.max)
                    nc.vector.tensor_tensor(out=XE[:, k, :], in0=xm1[:], in1=xm2[:], op=OP.subtract)
                    nc.vector.tensor_scalar(out=XE[:, k, :], in0=XE[:, k, :], scalar1=1.0, scalar2=0.0, op0=OP.add, op1=OP.max)

            GTABS = pool.tile([128, 12], F32, tag="gtabs")
            nc.sync.dma_start(GTABS[:], gtab[0:128, :])
            FB = pool.tile([128, T], F32)
            nc.gpsimd.dma_start(FB[:], flags[:].rearrange("(p f) -> p f", p=128))

            # ---------- main sweep + pipelined gather, 12 chunks ----------
            POS = pool.tile([128, T], F32)
            LAB = pool.tile([128, T], F32)
            GTH = pool.tile([128, T, 12], F32)
            CT = WLOC  # 25 tiles per chunk
            for d in range(NA):
                for it in range(NIT):
                    off = d * NIT + it
                    npart = 128 if it < NIT - 1 else H - 128 * (NIT - 1)
                    k6 = d * 6 + it
                    MXc = djp.tile([128, CT], F32, tag="MXc")
                    ACCc = djp.tile([128, CT], F32, tag="ACCc")
                    nc.any.memset(MXc[:], -1.0)
                    nc.any.memset(ACCc[:], 0.0)
                    for jl in range(WLOC):
                        jd = jl * 2 + d
                        R = djp.tile([128, 128], F32, tag="R")
                        nc.vector.tensor_tensor(out=R[:], in0=XE[:, k6, :], in1=YEB[:, jd * 128:(jd + 1) * 128], op=OP.mult)
                        nc.vector.tensor_reduce(out=MXc[:, jl:jl + 1], in_=R[:], axis=AX.X, op=OP.max)
                        junk = djp.tile([128, 128], F32, tag="junk")
                        nc.vector.scalar_tensor_tensor(
                            out=junk[:], in0=R[:], scalar=MXc[:, jl:jl + 1],
                            in1=RIOTA[:], op0=OP.is_ge, op1=OP.mult,
                            accum_out=ACCc[:, jl:jl + 1])
                    # ---- labels / pos for this chunk (t = jl*12 + off) ----
                    fbs = FB[:].rearrange("p (jl q) -> p jl q", q=12)[:, :, off]
                    poss = POS[:].rearrange("p (jl q) -> p jl q", q=12)[:, :, off]
                    labs = LAB[:].rearrange("p (jl q) -> p jl q", q=12)[:, :, off]
                    posa = djp.tile([128, CT], F32, tag="posa")
                    nc.vector.tensor_scalar(out=posa[:], in0=MXc[:], scalar1=POSR, scalar2=None, op0=OP.is_gt)
                    nc.vector.tensor_tensor(out=poss, in0=posa[:], in1=fbs, op=OP.max)
                    negc = djp.tile([128, CT], F32, tag="negc")
                    nc.vector.tensor_scalar(out=negc[:], in0=MXc[:], scalar1=NEGR, scalar2=None, op0=OP.is_lt)
                    x1 = djp.tile([128, CT], F32, tag="x1c")
                    nc.vector.tensor_tensor(out=x1[:], in0=negc[:], in1=poss, op=OP.mult)
                    nc.vector.tensor_tensor(out=x1[:], in0=negc[:], in1=x1[:], op=OP.subtract)
                    nc.vector.scalar_tensor_tensor(out=labs, in0=poss, scalar=2.0, in1=x1[:], op0=OP.mult, op1=OP.add)
                    nc.vector.tensor_scalar(out=labs, in0=labs, scalar1=-1.0, scalar2=None, op0=OP.add)
                    # ---- masked gather index ----
                    idxfc = djp.tile([128, CT], F32, tag="idxfc")
                    nc.vector.tensor_scalar(out=idxfc[:], in0=ACCc[:], scalar1=-1.0, scalar2=None, op0=OP.mult)
                    nc.vector.tensor_tensor(out=idxfc[:], in0=idxfc[:], in1=poss, op=OP.mult)
                    nc.vector.tensor_scalar(out=idxfc[:], in0=idxfc[:], scalar1=128.0, scalar2=None, op0=OP.add)
                    # ---- flatten -> DRAM, broadcast back, one-hot, PE gather ----
                    nc.sync.dma_start(idxrow[off, :].rearrange("(jl p) -> p jl", p=128), idxfc[:])
                    IDXB = djp.tile([128, CT * 128], F32, tag="IDXB")
                    nc.sync.dma_start(IDXB[:], idxrow[off:off + 1, :].to_broadcast([128, CT * 128]))
                    EQT = djp.tile([128, CT * 128], F32, tag="EQT")
                    nc.vector.tensor_scalar(out=EQT[:], in0=IDXB[:], scalar1=IOTAM[:], scalar2=None, op0=OP.is_equal)
                    for g0 in range(0, CT, 4):
                        gn = min(4, CT - g0)
                        gp = gpsum.tile([128, 48], F32, tag="gp")
                        for k in range(gn):
                            nc.tensor.matmul(gp[:, k * 12:(k + 1) * 12], lhsT=EQT[:, (g0 + k) * 128:(g0 + k + 1) * 128], rhs=GTABS[:], start=True, stop=True)
                        gdst = GTH[:].rearrange("p (jl q) c -> p jl q c", q=12)[:, g0:g0 + gn, off, :]
                        nc.scalar.copy(gdst, gp[:, 0:gn * 12].rearrange("p (jl c) -> p jl c", c=12))

            # ---------- encode adjustments (cols 0,1,6,7), pos-masked ----------
            for it in range(NIT):
                g0 = GTH[:, :, 0].rearrange("p (jd nit) -> p jd nit", nit=NIT)[:, :, it]
                ps = POS[:].rearrange("p (jd nit) -> p jd nit", nit=NIT)[:, :, it]
                nc.vector.scalar_tensor_tensor(out=g0, in0=g0, scalar=XADV[:, it:it + 1], in1=ps, op0=OP.subtract, op1=OP.mult)
            for jl in range(WLOC):
                g1 = GTH[:, jl * 12:(jl + 1) * 12, 1]
                ps = POS[:, jl * 12:(jl + 1) * 12]
                nc.vector.scalar_tensor_tensor(out=g1, in0=g1, scalar=YADV[:, jl:jl + 1], in1=ps, op0=OP.subtract, op1=OP.mult)
            for d in range(NA):
                for col in (6, 7):
                    g = GTH[:, :, col].rearrange("p (jl d nit) -> p jl d nit", d=NA, nit=NIT)[:, :, d, :]
                    ps = POS[:].rearrange("p (jl d nit) -> p jl d nit", d=NA, nit=NIT)[:, :, d, :]
                    cc = _TRIG[d][0] if col == 6 else _TRIG[d][1]
                    nc.vector.scalar_tensor_tensor(out=g, in0=g, scalar=float(cc), in1=ps, op0=OP.subtract, op1=OP.mult)

            # ---------- outputs ----------
            for it in range(NIT):
                np_ = 128 if it < NIT - 1 else H - 128 * (NIT - 1)
                lsrc = LAB[:].rearrange("p (jd nit) -> p jd nit", nit=NIT)[0:np_, :, it]
                nc.sync.dma_start(labels_o[:, it * 128:it * 128 + np_].rearrange("jd i -> i jd"), lsrc)
                gsrc = GTH[:].rearrange("p (jd nit) c -> p jd nit c", nit=NIT)[0:np_, :, it, 0:8]
                nc.sync.dma_start(reg_o[:, it * 128:it * 128 + np_, :].rearrange("jd i c -> i jd c"), gsrc)
                dsrc = GTH[:].rearrange("p (jd nit) c -> p jd nit c", nit=NIT)[0:np_, :, it, 8:12]
                nc.sync.dma_start(dir_o[:, it * 128:it * 128 + np_, :].rearrange("jd i c -> i jd c"), dsrc)

    if split:
        _split_excess_waits(nc)
    return nc


def _split_excess_waits(nc, limit=1):
    """This walrus build accepts only `limit` sync-waits per instruction.
    Peel extras onto preceding same-engine wait carriers."""
    for f in nc.m.functions:
        for bb in f.blocks:
            insts = list(bb.instructions)
            out, changed, k = [], False, 0
            for inst in insts:
                si = inst.sync_info
                if (si is not None and si.on_wait is not None
                        and len(si.on_wait) > limit):
                    waits = list(si.on_wait)
                    for w in waits[:-limit]:
                        nop = mybir.InstDrain(name=f"{inst.name}-wsp{k}")
                        k += 1
                        nop.engine = inst.engine
                        nop.sync_info = mybir.SyncInfo(on_wait=[w], on_update=[])
                        out.append(nop)
                    si.on_wait = waits[-limit:]
                    inst.sync_info = si
                    changed = True
                out.append(inst)
            if changed:
                bb.instructions = out


def _trig():
    rs = np.deg2rad(np.asarray([0.0, 90.0], np.float32))
    return [(np.cos(rs[d:d + 1])[0], np.sin(rs[d:d + 1])[0]) for d in range(NA)]


_TRIG = _trig()


def _host_inputs(gt_boxes, anchors, standup_anchors):
    f = np.float32
    gt = np.asarray(gt_boxes, f)
    su = np.asarray(standup_anchors, f)
    an = np.asarray(anchors, f)

    # gt standup boxes (reference boxes3d_to_standup_bboxes in f32)
    x, y = gt[:, 0], gt[:, 1]
    l, w, r = gt[:, 3], gt[:, 4], gt[:, 6]
    c, s = np.abs(np.cos(r)), np.abs(np.sin(r))
    ex = f(0.5) * (l * c + w * s)
    ey = f(0.5) * (l * s + w * c)
    gxmin, gymin, gxmax, gymax = x - ex, y - ey, x + ex, y + ey
    a2 = (gxmax - gxmin + f(1)) * (gymax - gymin + f(1))

    # anchor standup fields from the actual input (i: stride 400; j: stride 2)
    axmax = np.stack([su[np.arange(H) * (W * NA) + d, 2] for d in range(NA)])
    axmin = np.stack([su[np.arange(H) * (W * NA) + d, 0] for d in range(NA)])
    aymax = np.stack([su[np.arange(W) * NA + d, 3] for d in range(NA)])
    aymin = np.stack([su[np.arange(W) * NA + d, 1] for d in range(NA)])
    a1c = np.array([(su[d, 2] - su[d, 0] + f(1)) * (su[d, 3] - su[d, 1] + f(1))
                    for d in range(NA)], f)
    invs = np.stack([f(1) / (a1c[d] + a2) for d in range(NA)], axis=1)

    # encode table
    la, wa, ha = an[0, 3], an[0, 4], an[0, 5]
    diag = np.sqrt(la * la + wa * wa)
    zg, lg, wg, hg, rg = gt[:, 2], gt[:, 3], gt[:, 4], gt[:, 5], gt[:, 6]
    two_pi = f(2.0 * np.pi)
    half_pi = f(np.pi / 2.0)
    q = np.clip(np.floor(np.mod(rg, two_pi) / half_pi), 0, 3).astype(np.int32)
    oh = np.zeros((M, 4), f)
    oh[np.arange(M), q] = f(1)
    gtab = np.zeros((129, 12), f)
    gtab[:M, 0] = gt[:, 0] / diag
    gtab[:M, 1] = gt[:, 1] / diag
    gtab[:M, 2] = (zg - f(-1.0)) / ha
    gtab[:M, 3] = np.log(lg / la)
    gtab[:M, 4] = np.log(wg / wa)
    gtab[:M, 5] = np.log(hg / ha)
    gtab[:M, 6] = np.cos(rg)
    gtab[:M, 7] = np.sin(rg)
    gtab[:M, 8:12] = oh

    xa = an[np.arange(H) * (W * NA), 0]
    ya = an[np.arange(W) * NA, 1]

    rep = lambda row: np.broadcast_to(np.asarray(row, f)[None, :], (128, len(row))).copy()
    col = lambda v: np.asarray(v, f).reshape(-1, 1).copy()

    xadv = np.zeros((128, 6), f)
    for it in range(NIT):
        n = min(128, H - it * 128)
        xadv[:n, it] = xa[it * 128:it * 128 + n] / diag
    axmaxv_in = np.full((128, 12), f(-1e9))
    axminv_in = np.full((128, 12), f(1e9))
    for d in range(NA):
        for it in range(NIT):
            n = min(128, H - it * 128)
            axmaxv_in[:n, d * 6 + it] = axmax[d, it * 128:it * 128 + n]
            axminv_in[:n, d * 6 + it] = axmin[d, it * 128:it * 128 + n]

    glob = dict(
        gxmaxv=col(gxmax), gxminv=col(gxmin), gymaxv=col(gymax), gyminv=col(gymin),
        invsv=invs,
        axmaxb=rep(axmax.reshape(-1)), axminb=rep(axmin.reshape(-1)),
        aymaxb=rep(aymax.reshape(-1)), ayminb=rep(aymin.reshape(-1)),
        riota=rep(128.0 - np.arange(128)),
        ibig7=rep(np.arange(H) - BIGI),
        ibig2=rep(np.arange(W) - BIGI),
        gtab=gtab, xadv=xadv,
        iotam=np.arange(128, dtype=f).reshape(128, 1),
        axmaxv=axmaxv_in, axminv=axminv_in,
        gxmaxb=np.broadcast_to(gxmax[None, :], (128, 128)).copy(),
        gxminb=np.broadcast_to(gxmin[None, :], (128, 128)).copy(),
    )
    per_core = []
    for cidx in range(NC_):
        jlo = cidx * WLOC
        aylmax = np.zeros(JD, f)
        aylmin = np.zeros(JD, f)
        for jl in range(WLOC):
            for d in range(NA):
                aylmax[jl * 2 + d] = aymax[d, jlo + jl]
                aylmin[jl * 2 + d] = aymin[d, jlo + jl]
        m = dict(glob)
        m.update(aylmaxb=rep(aylmax), aylminb=rep(aylmin),
                 jlov=np.full((128, 1), f(jlo)),
                 yadv=rep(ya[jlo:jlo + WLOC] / diag))
        per_core.append(m)
    return per_core, None


def kernel(gt_boxes, anchors, standup_anchors):
    if "nc" not in _CACHE:
        _CACHE["nc"] = _build_nc()
    nc = _CACHE["nc"]

    per_core, _ = _host_inputs(gt_boxes, anchors, standup_anchors)
    res = run_bass_kernel_spmd(nc, per_core, list(range(NC_)))

    f = np.float32
    labels = np.empty((H, W, NA), f)
    reg = np.empty((H, W, NA, 8), f)
    dire = np.empty((H, W, NA, 4), f)
    for cidx in range(NC_):
        jlo = cidx * WLOC
        r = res.results[cidx]
        labels[:, jlo:jlo + WLOC, :] = r["labels_loc"].reshape(WLOC, NA, H).transpose(2, 0, 1)
        reg[:, jlo:jlo + WLOC, :, :] = r["reg_loc"].reshape(WLOC, NA, H, 8).transpose(2, 0, 1, 3)
        dire[:, jlo:jlo + WLOC, :, :] = r["dir_loc"].reshape(WLOC, NA, H, 4).transpose(2, 0, 1, 3)
    return labels.reshape(-1), reg.reshape(-1, 8), dire.reshape(-1, 4)


# revision 20
# speedup vs baseline: 1.6603x; 1.0761x over previous
"""BoxAnchorAssigner on 8 Trainium2 NeuronCores (Bass/Tile).

Strategy
--------
anchors form a regular (H=704, W=200, na=2) grid; the standup-IoU between an
anchor (i,j,d) and gt m separates:  inter = xe_d[i,m] * ye_d[j,m]  where
xe/ye are per-axis overlap extents.  With S[m] = a1 + a2[m],
iou = inter/(S-inter) = r/(1-r) monotone in r = inter/S, so ALL outputs
(thresholds, row argmax, column argmax "top1") can be computed in
R = xe * (ye*invS) space.

Sharding: the W=200 axis is split 25-per-core across 8 cores.  Each core
builds the tiny global xe/ye tables (so the per-gt top1 is available
locally without collectives) and sweeps only its own 50 (j,d) pairs x
6 i-tiles = 300 (128 anchor x 128 gt) tiles.

Per tile: PE computes R = xe_slice^T @ diag(ye'_jd) into PSUM (the
m-layout xe table is directly the transposed weights); the DVE then does
one tensor_reduce(max) and one scalar_tensor_tensor with accum_out that
computes sum((R>=Rmax)*(128-m)) = 128-argmax in a single pass.  Per-gt
encode rows (precomputed on host, 129x12, row 128 = zeros) are gathered
per-anchor with indirect DMA using the pos-masked argmax index; the top1
flags are delivered by an indirect-DMA scatter of the separable per-gt
argmax candidates.
"""

import ml_dtypes
import numpy as np

import concourse.bass as bass
import concourse.mybir as mybir
from concourse.tile import TileContext
from concourse.bass_utils import run_bass_kernel_spmd

F32 = mybir.dt.float32
U32 = mybir.dt.uint32
OP = mybir.AluOpType
AX = mybir.AxisListType

H, W, NA, M = 704, 200, 2, 128
A = H * W * NA
NC_ = 8
WLOC = W // NC_            # 25 j's per core
JD = WLOC * NA             # 50 (j,d) pairs per core
NIT = 6                    # i-tiles (704 -> 5x128 + 64)
T = JD * NIT               # 300 tiles per core
NLOC = T * 128             # 38400 padded anchors per core
BIGI = float(2 ** 20)

POSR = float(np.float32(0.6) / np.float32(1.6))       # 0.375
NEGR = float(np.float32(0.45) / np.float32(1.45))     # 0.310344...
BIGIDX = 9.0e6

_CACHE = {}


def _build_nc(split=True):
    nc = bass.Bass("TRN2", target_bir_lowering=False, debug=False, num_devices=NC_)

    def inp(name, shape, dt=F32):
        return nc.declare_dram_parameter(name, shape, dt, isOutput=False)

    # ---- inputs (global unless noted) ----------------------------------
    gxmaxv = inp("gxmaxv", [128, 1])       # gt standup fields, m on partitions
    gxminv = inp("gxminv", [128, 1])
    gymaxv = inp("gymaxv", [128, 1])
    gyminv = inp("gyminv", [128, 1])
    invsv = inp("invsv", [128, 2])         # 1/S per (m, d)
    axmaxb = inp("axmaxb", [128, NA * H])  # anchor x fields, replicated rows
    axminb = inp("axminb", [128, NA * H])
    aymaxb = inp("aymaxb", [128, NA * W])  # anchor y fields (global)
    ayminb = inp("ayminb", [128, NA * W])
    aylmaxb = inp("aylmaxb", [128, JD])    # per-core: local y fields per (jl,d)
    aylminb = inp("aylminb", [128, JD])
    axmaxv = inp("axmaxv", [128, 12])      # anchor x field per (p, d*6+it), pad -1e9
    axminv = inp("axminv", [128, 12])      # pad +1e9
    gxmaxb = inp("gxmaxb", [128, 128])     # gt x fields replicated across partitions
    gxminb = inp("gxminb", [128, 128])
    riota = inp("riota", [128, 128])       # 128 - m  along free
    ibig7 = inp("ibig7", [128, H])         # i - 2^20 along free
    ibig2 = inp("ibig2", [128, W])         # j - 2^20 along free
    jlov = inp("jlov", [128, 1])           # per-core: global j base
    xadv = inp("xadv", [128, 6])           # xa/diag per (p, it), pad 0
    yadv = inp("yadv", [128, WLOC])        # per-core: ya/diag per jl
    gtab3 = inp("gtab3", [128, 36], mybir.dt.bfloat16)  # per-gt encode, 3-way bf16 split
    iotam = inp("iotam", [128, 1])         # m per partition

    labels_o = nc.declare_dram_parameter("labels_loc", [JD, H], F32, isOutput=True)
    reg_o = nc.declare_dram_parameter("reg_loc", [JD, H, 8], F32, isOutput=True)
    dir_o = nc.declare_dram_parameter("dir_loc", [JD, H, 4], F32, isOutput=True)

    flags = nc.dram_tensor("t1flags", [NLOC], F32)
    idxrow = nc.dram_tensor("idxrow", [NA * NIT, WLOC * 128], mybir.dt.bfloat16)

    with TileContext(nc) as tc:
        with tc.tile_pool(name="main", bufs=1) as pool, \
             tc.tile_pool(name="dj", bufs=2) as djp, \
             tc.tile_pool(name="gps", bufs=4, space="PSUM") as gpsum:

            def load(ap_in, shape, dt=F32):
                t = pool.tile(shape, dt, tag=ap_in.name)
                nc.sync.dma_start(t[:], ap_in[:])
                return t

            GXMAXV = load(gxmaxv, [128, 1]); GXMINV = load(gxminv, [128, 1])
            GYMAXV = load(gymaxv, [128, 1]); GYMINV = load(gyminv, [128, 1])
            INVSV = load(invsv, [128, 2])
            AXMAXB = load(axmaxb, [128, NA * H]); AXMINB = load(axminb, [128, NA * H])
            AYMAXB = load(aymaxb, [128, NA * W]); AYMINB = load(ayminb, [128, NA * W])
            AYLMAXB = load(aylmaxb, [128, JD]); AYLMINB = load(aylminb, [128, JD])
            AXMAXV = load(axmaxv, [128, 12]); AXMINV = load(axminv, [128, 12])
            GXMAXB = load(gxmaxb, [128, 128]); GXMINB = load(gxminb, [128, 128])
            RIOTA = load(riota, [128, 128])
            IBIG7 = load(ibig7, [128, H]); IBIG2 = load(ibig2, [128, W])
            JLOV = load(jlov, [128, 1])
            XADV = load(xadv, [128, 6]); YADV = load(yadv, [128, WLOC])
            IOTAM = load(iotam, [128, 1])

            # ---------- m-layout extent tables ----------
            def ext_table(amax_ap, amin_ap, gmaxv, gminv, n, inv_d, tag):
                m1 = pool.tile([128, n], F32, tag=f"m1_{tag}")
                nc.vector.tensor_scalar(out=m1[:], in0=amax_ap, scalar1=gmaxv[:], scalar2=None, op0=OP.min)
                m2 = pool.tile([128, n], F32, tag=f"m2_{tag}")
                nc.vector.tensor_scalar(out=m2[:], in0=amin_ap, scalar1=gminv[:], scalar2=None, op0=OP.max)
                t1 = pool.tile([128, n], F32, tag=f"t_{tag}")
                nc.vector.tensor_tensor(out=t1[:], in0=m1[:], in1=m2[:], op=OP.subtract)
                nc.vector.tensor_scalar(out=t1[:], in0=t1[:], scalar1=1.0, scalar2=0.0, op0=OP.add, op1=OP.max)
                if inv_d is not None:
                    nc.vector.tensor_scalar(out=t1[:], in0=t1[:], scalar1=INVSV[:, inv_d:inv_d + 1], scalar2=None, op0=OP.mult)
                return t1

            # xe (raw, used as PE weights + for top1), ye' (with invS)
            XEM = [ext_table(AXMAXB[:, d * H:(d + 1) * H], AXMINB[:, d * H:(d + 1) * H],
                             GXMAXV, GXMINV, H, None, f"xe{d}") for d in range(NA)]
            YEM = [ext_table(AYMAXB[:, d * W:(d + 1) * W], AYMINB[:, d * W:(d + 1) * W],
                             GYMAXV, GYMINV, W, d, f"ye{d}") for d in range(NA)]
            # local ye' columns (bit-identical recompute on the local slice)
            YEL = ext_table(AYLMAXB[:], AYLMINB[:], GYMAXV, GYMINV, JD, None, "yel")
            for d in range(NA):
                nc.vector.tensor_scalar(out=YEL[:, d::NA], in0=YEL[:, d::NA], scalar1=INVSV[:, d:d + 1], scalar2=None, op0=OP.mult)

            # ---------- top1 (separable, global) ----------
            # first-argmax + max per (m, d) over i and j
            def fam(table, n, ibig, tag):
                mxv = pool.tile([128, 1], F32, tag=f"mx_{tag}")
                nc.vector.tensor_reduce(out=mxv[:], in_=table[:], axis=AX.X, op=OP.max)
                junk = pool.tile([128, n], F32, tag=f"jk_{tag}")
                mn = pool.tile([128, 1], F32, tag=f"mn_{tag}")
                nc.vector.scalar_tensor_tensor(out=junk[:], in0=table[:], scalar=mxv[:], in1=ibig, op0=OP.is_ge, op1=OP.mult, accum_out=None)
                nc.vector.tensor_reduce(out=mn[:], in_=junk[:], axis=AX.X, op=OP.min)
                idx = pool.tile([128, 1], F32, tag=f"ix_{tag}")
                nc.vector.tensor_scalar(out=idx[:], in0=mn[:], scalar1=BIGI, scalar2=None, op0=OP.add)
                return mxv, idx

            mxs, ixs, mys, jxs = [], [], [], []
            for d in range(NA):
                a, b = fam(XEM[d], H, IBIG7[:], f"x{d}")
                mxs.append(a); ixs.append(b)
                a, b = fam(YEM[d], W, IBIG2[:], f"y{d}")
                mys.append(a); jxs.append(b)

            def tiny(op, a, b, tag):
                t = pool.tile([128, 1], F32, tag=tag)
                nc.vector.tensor_tensor(out=t[:], in0=a[:], in1=b[:], op=op)
                return t

            P = [tiny(OP.mult, mxs[d], mys[d], f"P{d}") for d in range(NA)]
            gt1 = tiny(OP.is_gt, P[1], P[0], "gt1")
            eqp = tiny(OP.is_equal, P[1], P[0], "eqp")
            li = tiny(OP.is_lt, ixs[1], ixs[0], "li")
            ei = tiny(OP.is_equal, ixs[1], ixs[0], "ei")
            lj = tiny(OP.is_lt, jxs[1], jxs[0], "lj")
            t_a = tiny(OP.min, ei, lj, "t_a")
            tlt = tiny(OP.max, li, t_a, "tlt")
            t_b = tiny(OP.min, eqp, tlt, "t_b")
            use1 = tiny(OP.max, gt1, t_b, "use1")

            def blend(a1, a0, mask, tag):  # mask ? a1 : a0
                t1 = tiny(OP.subtract, a1, a0, tag + "_d")
                t2 = tiny(OP.mult, mask, t1, tag + "_m")
                return tiny(OP.add, a0, t2, tag)

            iT = blend(ixs[1], ixs[0], use1, "iT")
            jT = blend(jxs[1], jxs[0], use1, "jT")

            # Lidx = (iT%128)*T + ((jT-jlo)*2 + dT)*6 + iT//128 ; dT == use1
            iu = pool.tile([128, 1], U32, tag="iu")
            nc.vector.tensor_copy(iu[:], iT[:])
            pmu = pool.tile([128, 1], U32, tag="pmu")
            nc.vector.tensor_scalar(out=pmu[:], in0=iu[:], scalar1=127, scalar2=None, op0=OP.bitwise_and)
            itu = pool.tile([128, 1], U32, tag="itu")
            nc.vector.tensor_scalar(out=itu[:], in0=iu[:], scalar1=7, scalar2=None, op0=OP.logical_shift_right)
            pmf = pool.tile([128, 1], F32, tag="pmf")
            nc.vector.tensor_copy(pmf[:], pmu[:])
            itf = pool.tile([128, 1], F32, tag="itf")
            nc.vector.tensor_copy(itf[:], itu[:])
            l1 = tiny(OP.subtract, jT, JLOV, "l1")
            l2 = pool.tile([128, 1], F32, tag="l2")
            nc.vector.scalar_tensor_tensor(out=l2[:], in0=l1[:], scalar=2.0, in1=use1[:], op0=OP.mult, op1=OP.add)
            l2b = pool.tile([128, 1], F32, tag="l2b")
            nc.vector.scalar_tensor_tensor(out=l2b[:], in0=l2[:], scalar=6.0, in1=itf[:], op0=OP.mult, op1=OP.add)
            l3 = pool.tile([128, 1], F32, tag="l3")
            nc.vector.scalar_tensor_tensor(out=l3[:], in0=pmf[:], scalar=float(T), in1=l2b[:], op0=OP.mult, op1=OP.add)
            jhi = pool.tile([128, 1], F32, tag="jhi")
            nc.vector.tensor_scalar(out=jhi[:], in0=JLOV[:], scalar1=float(WLOC), scalar2=None, op0=OP.add)
            v1 = tiny(OP.is_ge, jT, JLOV, "v1")
            v2 = tiny(OP.is_lt, jT, jhi, "v2")
            vld = tiny(OP.min, v1, v2, "vld")
            big = pool.tile([128, 1], F32, tag="big")
            nc.any.memset(big[:], BIGIDX)
            lfin = blend(l3, big, vld, "lfin")
            lu = pool.tile([128, 1], U32, tag="lu")
            nc.vector.tensor_copy(lu[:], lfin[:])

            zt = pool.tile([128, T], F32, tag="zt")
            nc.any.memset(zt[:], 0.0)
            nc.sync.dma_start(flags[:].rearrange("(p f) -> p f", p=128), zt[:])
            onet = pool.tile([128, 1], F32, tag="onet")
            nc.any.memset(onet[:], 1.0)
            nc.gpsimd.indirect_dma_start(
                out=flags[:, None], out_offset=bass.IndirectOffsetOnAxis(ap=lu[:], axis=0),
                in_=onet[:], in_offset=None, bounds_check=NLOC - 1, oob_is_err=False)

            # ---------- ye rows -> DRAM -> broadcast YEB ----------
            ytd = nc.dram_tensor("ytd", [JD, 128], F32)
            nc.sync.dma_start(ytd[:].rearrange("jd m -> m jd"), YEL[:])
            YEB = pool.tile([128, JD * 128], F32)
            nc.sync.dma_start(YEB[:], ytd[:].rearrange("jd m -> (jd m)")[None, :].to_broadcast([128, JD * 128]))

            # ---------- XE tiles in (i-part, m-free) layout ----------
            XE = pool.tile([128, 12, 128], F32)
            for d in range(NA):
                for it in range(NIT):
                    k = d * 6 + it
                    xm1 = djp.tile([128, 128], F32, tag="xm1")
                    nc.vector.tensor_scalar(out=xm1[:], in0=GXMAXB[:], scalar1=AXMAXV[:, k:k + 1], scalar2=None, op0=OP.min)
                    xm2 = djp.tile([128, 128], F32, tag="xm2")
                    nc.vector.tensor_scalar(out=xm2[:], in0=GXMINB[:], scalar1=AXMINV[:, k:k + 1], scalar2=None, op0=OP.max)
                    nc.vector.tensor_tensor(out=XE[:, k, :], in0=xm1[:], in1=xm2[:], op=OP.subtract)
                    nc.vector.tensor_scalar(out=XE[:, k, :], in0=XE[:, k, :], scalar1=1.0, scalar2=0.0, op0=OP.add, op1=OP.max)

            GTABS = pool.tile([128, 36], mybir.dt.bfloat16, tag="gtabs")
            nc.sync.dma_start(GTABS[:], gtab3[:])
            FB = pool.tile([128, T], F32)
            nc.gpsimd.dma_start(FB[:], flags[:].rearrange("(p f) -> p f", p=128))

            # ---------- main sweep + pipelined gather, 12 chunks ----------
            POS = pool.tile([128, T], F32)
            LAB = pool.tile([128, T], F32)
            GTH = pool.tile([128, T, 12], F32)
            CT = WLOC  # 25 tiles per chunk
            for d in range(NA):
                for it in range(NIT):
                    off = d * NIT + it
                    npart = 128 if it < NIT - 1 else H - 128 * (NIT - 1)
                    k6 = d * 6 + it
                    MXc = djp.tile([128, CT], F32, tag="MXc")
                    ACCc = djp.tile([128, CT], F32, tag="ACCc")
                    nc.any.memset(MXc[:], -1.0)
                    nc.any.memset(ACCc[:], 0.0)
                    Rb = djp.tile([128, CT, 128], F32, tag="Rb")
                    yebs = YEB[:].rearrange("p (jl dd m) -> p jl dd m", dd=NA, m=128)[:, :, d, :]
                    nc.vector.tensor_tensor(out=Rb[:], in0=XE[:, k6:k6 + 1, :].to_broadcast([128, CT, 128]), in1=yebs, op=OP.mult)
                    nc.vector.tensor_reduce(out=MXc[:], in_=Rb[:], axis=AX.X, op=OP.max)
                    for jl in range(WLOC):
                        junk = djp.tile([128, 128], F32, tag="junk")
                        nc.vector.scalar_tensor_tensor(
                            out=junk[:], in0=Rb[:, jl, :], scalar=MXc[:, jl:jl + 1],
                            in1=RIOTA[:], op0=OP.is_ge, op1=OP.mult,
                            accum_out=ACCc[:, jl:jl + 1])
                    # ---- labels / pos for this chunk (t = jl*12 + off) ----
                    fbs = FB[:].rearrange("p (jl q) -> p jl q", q=12)[:, :, off]
                    poss = POS[:].rearrange("p (jl q) -> p jl q", q=12)[:, :, off]
                    labs = LAB[:].rearrange("p (jl q) -> p jl q", q=12)[:, :, off]
                    posa = djp.tile([128, CT], F32, tag="posa")
                    nc.vector.tensor_scalar(out=posa[:], in0=MXc[:], scalar1=POSR, scalar2=None, op0=OP.is_gt)
                    nc.vector.tensor_tensor(out=poss, in0=posa[:], in1=fbs, op=OP.max)
                    negc = djp.tile([128, CT], F32, tag="negc")
                    nc.vector.tensor_scalar(out=negc[:], in0=MXc[:], scalar1=NEGR, scalar2=None, op0=OP.is_lt)
                    x1 = djp.tile([128, CT], F32, tag="x1c")
                    nc.vector.tensor_tensor(out=x1[:], in0=negc[:], in1=poss, op=OP.mult)
                    nc.vector.tensor_tensor(out=x1[:], in0=negc[:], in1=x1[:], op=OP.subtract)
                    nc.vector.scalar_tensor_tensor(out=labs, in0=poss, scalar=2.0, in1=x1[:], op0=OP.mult, op1=OP.add)
                    nc.vector.tensor_scalar(out=labs, in0=labs, scalar1=-1.0, scalar2=None, op0=OP.add)
                    # ---- masked gather index ----
                    idxfc = djp.tile([128, CT], F32, tag="idxfc")
                    idxfb = djp.tile([128, CT], mybir.dt.bfloat16, tag="idxfb")
                    nc.vector.tensor_scalar(out=idxfc[:], in0=ACCc[:], scalar1=-1.0, scalar2=None, op0=OP.mult)
                    nc.vector.tensor_tensor(out=idxfc[:], in0=idxfc[:], in1=poss, op=OP.mult)
                    nc.vector.tensor_scalar(out=idxfb[:], in0=idxfc[:], scalar1=128.0, scalar2=None, op0=OP.add)
                    # ---- flatten -> DRAM, broadcast back, one-hot, PE gather ----
                    nc.sync.dma_start(idxrow[off, :].rearrange("(jl p) -> p jl", p=128), idxfb[:])
                    IDXB = djp.tile([128, CT * 128], mybir.dt.bfloat16, tag="IDXB")
                    nc.sync.dma_start(IDXB[:], idxrow[off:off + 1, :].to_broadcast([128, CT * 128]))
                    EQT = djp.tile([128, CT * 128], mybir.dt.bfloat16, tag="EQT")
                    nc.vector.tensor_scalar(out=EQT[:], in0=IDXB[:], scalar1=IOTAM[:], scalar2=None, op0=OP.is_equal)
                    for g0 in range(0, CT, 4):
                        gn = min(4, CT - g0)
                        gp = gpsum.tile([128, 48], F32, tag="gp")
                        for k in range(gn):
                            lhsw = EQT[:, (g0 + k) * 128:(g0 + k + 1) * 128]
                            nc.tensor.matmul(gp[:, k * 12:(k + 1) * 12], lhsT=lhsw, rhs=GTABS[:, 0:12], start=True, stop=False)
                            nc.tensor.matmul(gp[:, k * 12:(k + 1) * 12], lhsT=lhsw, rhs=GTABS[:, 12:24], start=False, stop=False)
                            nc.tensor.matmul(gp[:, k * 12:(k + 1) * 12], lhsT=lhsw, rhs=GTABS[:, 24:36], start=False, stop=True)
                        gdst = GTH[:].rearrange("p (jl q) c -> p jl q c", q=12)[:, g0:g0 + gn, off, :]
                        nc.scalar.copy(gdst, gp[:, 0:gn * 12].rearrange("p (jl c) -> p jl c", c=12))

            # ---------- encode adjustments (cols 0,1,6,7), pos-masked ----------
            for it in range(NIT):
                g0 = GTH[:, :, 0].rearrange("p (jd nit) -> p jd nit", nit=NIT)[:, :, it]
                ps = POS[:].rearrange("p (jd nit) -> p jd nit", nit=NIT)[:, :, it]
                nc.vector.scalar_tensor_tensor(out=g0, in0=g0, scalar=XADV[:, it:it + 1], in1=ps, op0=OP.subtract, op1=OP.mult)
            for jl in range(WLOC):
                g1 = GTH[:, jl * 12:(jl + 1) * 12, 1]
                ps = POS[:, jl * 12:(jl + 1) * 12]
                nc.vector.scalar_tensor_tensor(out=g1, in0=g1, scalar=YADV[:, jl:jl + 1], in1=ps, op0=OP.subtract, op1=OP.mult)
            for d in range(NA):
                for col in (6, 7):
                    g = GTH[:, :, col].rearrange("p (jl d nit) -> p jl d nit", d=NA, nit=NIT)[:, :, d, :]
                    ps = POS[:].rearrange("p (jl d nit) -> p jl d nit", d=NA, nit=NIT)[:, :, d, :]
                    cc = _TRIG[d][0] if col == 6 else _TRIG[d][1]
                    nc.vector.scalar_tensor_tensor(out=g, in0=g, scalar=float(cc), in1=ps, op0=OP.subtract, op1=OP.mult)

            # ---------- outputs ----------
            for it in range(NIT):
                np_ = 128 if it < NIT - 1 else H - 128 * (NIT - 1)
                lsrc = LAB[:].rearrange("p (jd nit) -> p jd nit", nit=NIT)[0:np_, :, it]
                nc.sync.dma_start(labels_o[:, it * 128:it * 128 + np_].rearrange("jd i -> i jd"), lsrc)
                gsrc = GTH[:].rearrange("p (jd nit) c -> p jd nit c", nit=NIT)[0:np_, :, it, 0:8]
                nc.sync.dma_start(reg_o[:, it * 128:it * 128 + np_, :].rearrange("jd i c -> i jd c"), gsrc)
                dsrc = GTH[:].rearrange("p (jd nit) c -> p jd nit c", nit=NIT)[0:np_, :, it, 8:12]
                nc.sync.dma_start(dir_o[:, it * 128:it * 128 + np_, :].rearrange("jd i c -> i jd c"), dsrc)

    if split:
        _split_excess_waits(nc)
    return nc


def _split_excess_waits(nc, limit=1):
    """This walrus build accepts only `limit` sync-waits per instruction.
    Peel extras onto preceding same-engine wait carriers."""
    for f in nc.m.functions:
        for bb in f.blocks:
            insts = list(bb.instructions)
            out, changed, k = [], False, 0
            for inst in insts:
                si = inst.sync_info
                if (si is not None and si.on_wait is not None
                        and len(si.on_wait) > limit):
                    waits = list(si.on_wait)
                    for w in waits[:-limit]:
                        nop = mybir.InstDrain(name=f"{inst.name}-wsp{k}")
                        k += 1
                        nop.engine = inst.engine
                        nop.sync_info = mybir.SyncInfo(on_wait=[w], on_update=[])
                        out.append(nop)
                    si.on_wait = waits[-limit:]
                    inst.sync_info = si
                    changed = True
                out.append(inst)
            if changed:
                bb.instructions = out


def _trig():
    rs = np.deg2rad(np.asarray([0.0, 90.0], np.float32))
    return [(np.cos(rs[d:d + 1])[0], np.sin(rs[d:d + 1])[0]) for d in range(NA)]


_TRIG = _trig()


def _host_inputs(gt_boxes, anchors, standup_anchors):
    f = np.float32
    gt = np.asarray(gt_boxes, f)
    su = np.asarray(standup_anchors, f)
    an = np.asarray(anchors, f)

    # gt standup boxes (reference boxes3d_to_standup_bboxes in f32)
    x, y = gt[:, 0], gt[:, 1]
    l, w, r = gt[:, 3], gt[:, 4], gt[:, 6]
    c, s = np.abs(np.cos(r)), np.abs(np.sin(r))
    ex = f(0.5) * (l * c + w * s)
    ey = f(0.5) * (l * s + w * c)
    gxmin, gymin, gxmax, gymax = x - ex, y - ey, x + ex, y + ey
    a2 = (gxmax - gxmin + f(1)) * (gymax - gymin + f(1))

    # anchor standup fields from the actual input (i: stride 400; j: stride 2)
    axmax = np.stack([su[np.arange(H) * (W * NA) + d, 2] for d in range(NA)])
    axmin = np.stack([su[np.arange(H) * (W * NA) + d, 0] for d in range(NA)])
    aymax = np.stack([su[np.arange(W) * NA + d, 3] for d in range(NA)])
    aymin = np.stack([su[np.arange(W) * NA + d, 1] for d in range(NA)])
    a1c = np.array([(su[d, 2] - su[d, 0] + f(1)) * (su[d, 3] - su[d, 1] + f(1))
                    for d in range(NA)], f)
    invs = np.stack([f(1) / (a1c[d] + a2) for d in range(NA)], axis=1)

    # encode table
    la, wa, ha = an[0, 3], an[0, 4], an[0, 5]
    diag = np.sqrt(la * la + wa * wa)
    zg, lg, wg, hg, rg = gt[:, 2], gt[:, 3], gt[:, 4], gt[:, 5], gt[:, 6]
    two_pi = f(2.0 * np.pi)
    half_pi = f(np.pi / 2.0)
    q = np.clip(np.floor(np.mod(rg, two_pi) / half_pi), 0, 3).astype(np.int32)
    oh = np.zeros((M, 4), f)
    oh[np.arange(M), q] = f(1)
    gtab = np.zeros((128, 12), f)
    gtab[:M, 0] = gt[:, 0] / diag
    gtab[:M, 1] = gt[:, 1] / diag
    gtab[:M, 2] = (zg - f(-1.0)) / ha
    gtab[:M, 3] = np.log(lg / la)
    gtab[:M, 4] = np.log(wg / wa)
    gtab[:M, 5] = np.log(hg / ha)
    gtab[:M, 6] = np.cos(rg)
    gtab[:M, 7] = np.sin(rg)
    gtab[:M, 8:12] = oh
    bf = ml_dtypes.bfloat16
    g1 = gtab.astype(bf)
    r1 = gtab - g1.astype(f)
    g2 = r1.astype(bf)
    g3 = (r1 - g2.astype(f)).astype(bf)
    gtab3 = np.concatenate([g1, g2, g3], axis=1)

    xa = an[np.arange(H) * (W * NA), 0]
    ya = an[np.arange(W) * NA, 1]

    rep = lambda row: np.broadcast_to(np.asarray(row, f)[None, :], (128, len(row))).copy()
    col = lambda v: np.asarray(v, f).reshape(-1, 1).copy()

    xadv = np.zeros((128, 6), f)
    for it in range(NIT):
        n = min(128, H - it * 128)
        xadv[:n, it] = xa[it * 128:it * 128 + n] / diag
    axmaxv_in = np.full((128, 12), f(-1e9))
    axminv_in = np.full((128, 12), f(1e9))
    for d in range(NA):
        for it in range(NIT):
            n = min(128, H - it * 128)
            axmaxv_in[:n, d * 6 + it] = axmax[d, it * 128:it * 128 + n]
            axminv_in[:n, d * 6 + it] = axmin[d, it * 128:it * 128 + n]

    glob = dict(
        gxmaxv=col(gxmax), gxminv=col(gxmin), gymaxv=col(gymax), gyminv=col(gymin),
        invsv=invs,
        axmaxb=rep(axmax.reshape(-1)), axminb=rep(axmin.reshape(-1)),
        aymaxb=rep(aymax.reshape(-1)), ayminb=rep(aymin.reshape(-1)),
        riota=rep(128.0 - np.arange(128)),
        ibig7=rep(np.arange(H) - BIGI),
        ibig2=rep(np.arange(W) - BIGI),
        gtab3=gtab3, xadv=xadv,
        iotam=np.arange(128, dtype=f).reshape(128, 1),
        axmaxv=axmaxv_in, axminv=axminv_in,
        gxmaxb=np.broadcast_to(gxmax[None, :], (128, 128)).copy(),
        gxminb=np.broadcast_to(gxmin[None, :], (128, 128)).copy(),
    )
    per_core = []
    for cidx in range(NC_):
        jlo = cidx * WLOC
        aylmax = np.zeros(JD, f)
        aylmin = np.zeros(JD, f)
        for jl in range(WLOC):
            for d in range(NA):
                aylmax[jl * 2 + d] = aymax[d, jlo + jl]
                aylmin[jl * 2 + d] = aymin[d, jlo + jl]
        m = dict(glob)
        m.update(aylmaxb=rep(aylmax), aylminb=rep(aylmin),
                 jlov=np.full((128, 1), f(jlo)),
                 yadv=rep(ya[jlo:jlo + WLOC] / diag))
        per_core.append(m)
    return per_core, None


def kernel(gt_boxes, anchors, standup_anchors):
    if "nc" not in _CACHE:
        _CACHE["nc"] = _build_nc()
    nc = _CACHE["nc"]

    per_core, _ = _host_inputs(gt_boxes, anchors, standup_anchors)
    res = run_bass_kernel_spmd(nc, per_core, list(range(NC_)))

    f = np.float32
    labels = np.empty((H, W, NA), f)
    reg = np.empty((H, W, NA, 8), f)
    dire = np.empty((H, W, NA, 4), f)
    for cidx in range(NC_):
        jlo = cidx * WLOC
        r = res.results[cidx]
        labels[:, jlo:jlo + WLOC, :] = r["labels_loc"].reshape(WLOC, NA, H).transpose(2, 0, 1)
        reg[:, jlo:jlo + WLOC, :, :] = r["reg_loc"].reshape(WLOC, NA, H, 8).transpose(2, 0, 1, 3)
        dire[:, jlo:jlo + WLOC, :, :] = r["dir_loc"].reshape(WLOC, NA, H, 4).transpose(2, 0, 1, 3)
    return labels.reshape(-1), reg.reshape(-1, 8), dire.reshape(-1, 4)
